# revision 1
# baseline (speedup 1.0000x reference)
"""TRN2 Bass kernel: MultiHeadSelfAttention (B=4, S=2048, D=1024, H=16, DK=64).

Sharding: 8 cores = 4 batches x 2 head-groups (8 heads each).
Per core: QK path in float32r (TF32-ish, 1 cyc/row), V/P path bf16,
softmax via reduce_max + ACT exp(bias=-max), P^T via DMA-transpose (xbar),
PV with [V|1]-stationary -> [O^T; denom], 1/denom broadcast via gpsimd
partition_broadcast, normalization fused into the O^T eviction multiply,
output projection from O^T, partial Y out.
Host: pre-mask x (zeroed masked rows -> masked keys get score 0 -> exp
underflows to exact 0 like the reference's -1e6), pre-transpose x,
permute W columns to [head][dk], fold 1/sqrt(DK) into WQ; final
abs((Y0+Y1)*mask) on host after summing the two head-group partials.
"""

import os
import numpy as np

B, S, D, H, DK = 4, 2048, 1024, 16, 64
HG = 2            # head groups (tensor-parallel)
HL = H // HG      # heads per core = 8
DH = HL * DK      # 512 per-core head width
KT = D // 128     # 8 contraction tiles
NQ = S // 128     # 16 q tiles
NKC = S // 128    # 16 key chunks
QB = 4            # q blocks
QBW = S // QB     # 512 q block width

_cache = {}


def _build():
    from concourse import bacc
    import concourse.mybir as mybir
    import concourse.tile as tile
    from concourse.masks import make_identity

    f32 = mybir.dt.float32
    f32r = mybir.dt.float32r
    bf16 = mybir.dt.bfloat16
    Exp = mybir.ActivationFunctionType.Exp
    AX = mybir.AxisListType.X

    nc = bacc.Bacc("TRN2", target_bir_lowering=False, debug=False, num_devices=8)

    xT_d = nc.dram_tensor("xT", [D, S], f32, kind="ExternalInput")
    wq_d = nc.dram_tensor("wq", [D, DH], f32, kind="ExternalInput")
    wk_d = nc.dram_tensor("wk", [D, DH], f32, kind="ExternalInput")
    wv_d = nc.dram_tensor("wv", [D, DH], f32, kind="ExternalInput")
    wo_d = nc.dram_tensor("wo", [DH, D], f32, kind="ExternalInput")
    y_d = nc.dram_tensor("y", [S, D], f32, kind="ExternalOutput")

    with tile.TileContext(nc) as tc:
        with (
            tc.tile_pool(name="persist", bufs=1) as pp,
            tc.tile_pool(name="psA", bufs=int(os.environ.get("PSA", "7")), space="PSUM") as psA,
            tc.tile_pool(name="psC", bufs=1, space="PSUM") as psC,
        ):
            qT = pp.tile([128, KT // 2, S], f32r, tag="qT")   # (512,2048) 4 ptiles
            kT = pp.tile([128, KT // 2, S], f32r, tag="kT")
            # V with a ones column per head: blocks of 66 = [V_h(64) | 1 | pad]
            v_sb = pp.tile([128, NKC, HL, 66], bf16, tag="v")
            nc.gpsimd.memset(v_sb[:, :, :, 64:65], 1.0)
            wor = pp.tile([128, 4, D], f32r, tag="wor")
            nc.gpsimd.dma_start(wor[:], wo_d.rearrange("(t p) n -> p t n", p=128))

            # ---- phase 1: projections ----
            with (
                tc.tile_pool(name="ph1x", bufs=1) as px,
                tc.tile_pool(name="ph1w", bufs=10) as pw,
                tc.tile_pool(name="ph1wv", bufs=1) as pwv,
            ):
                xr = px.tile([128, KT, S], f32r, tag="xr")
                nc.gpsimd.dma_start(
                    xr[:], xT_d.rearrange("(t p) s -> p t s", p=128)
                )
                wvr = pwv.tile([128, KT, DH], f32r, tag="wvr")
                nc.gpsimd.dma_start(
                    wvr[:], wv_d.rearrange("(t p) n -> p t n", p=128)
                )
                for w_d, dst in ((wq_d, qT), (wk_d, kT)):
                    for p in range(4):
                        wchs = []
                        for k in range(KT):
                            wch = pw.tile([128, 128], f32r, tag="wch")
                            nc.gpsimd.dma_start(
                                wch[:],
                                w_d[k * 128:(k + 1) * 128, p * 128:(p + 1) * 128],
                            )
                            wchs.append(wch)
                        for n in range(4):
                            ps = psA.tile([128, 512], f32, tag="mm")
                            for k in range(KT):
                                nc.tensor.matmul(
                                    ps[:],
                                    wchs[k][:],
                                    xr[:, k, n * 512:(n + 1) * 512],
                                    start=(k == 0),
                                    stop=(k == KT - 1),
                                )
                            nc.vector.tensor_copy(
                                dst[:, p, n * 512:(n + 1) * 512], ps[:]
                            )
                for sc in range(NKC):
                    psv = psA.tile([128, 512], f32, tag="mm")
                    for k in range(KT):
                        nc.tensor.matmul(
                            psv[:],
                            xr[:, k, sc * 128:(sc + 1) * 128],
                            wvr[:, k, :],
                            start=(k == 0),
                            stop=(k == KT - 1),
                        )
                    nc.vector.tensor_copy(
                        v_sb[:, sc, :, 0:64],
                        psv[:].rearrange("p (h w) -> p h w", w=64),
                    )

            # ---- phase 2: attention + output projection ----
            with (
                tc.tile_pool(name="ptb", bufs=int(os.environ.get("PTB", "2")), space="SBUF") as ptbp,
                tc.tile_pool(name="pexp", bufs=int(os.environ.get("PEXP", "3"))) as pexp,
                tc.tile_pool(name="stats", bufs=4) as st,
                tc.tile_pool(name="oTp", bufs=2) as oTp,
                tc.tile_pool(name="yp", bufs=3) as yp,
            ):
                for qb in range(QB):
                    oT = oTp.tile([128, 4, QBW], f32r, tag="oT")
                    for hh in range(HL):
                        p, r0 = hh // 2, (hh % 2) * 64
                        ptb = ptbp.tile([128, QBW // 128, NKC, 128], bf16, tag="ptb")
                        for il in range(QBW // 128):
                            i = qb * 4 + il
                            sq = []
                            for n in range(4):
                                t = psA.tile([128, 512], f32, tag="mm")
                                nc.tensor.matmul(
                                    t[:],
                                    qT[r0:r0 + DK, p, i * 128:(i + 1) * 128],
                                    kT[r0:r0 + DK, p, n * 512:(n + 1) * 512],
                                    start=True,
                                    stop=True,
                                )
                                sq.append(t)
                            mx4 = st.tile([128, 4], f32, tag="mx4")
                            for n in range(4):
                                nc.vector.reduce_max(
                                    mx4[:, n:n + 1], sq[n][:], axis=AX
                                )
                            nm = st.tile([128, 1], f32, tag="nm")
                            nc.vector.tensor_reduce(
                                nm[:], mx4[:], axis=AX,
                                op=mybir.AluOpType.max, negate=True,
                            )
                            p_sb = pexp.tile([128, S], bf16, tag="p")
                            for n in range(4):
                                nc.scalar.activation(
                                    p_sb[:, n * 512:(n + 1) * 512],
                                    sq[n][:],
                                    Exp,
                                    bias=nm[:],
                                    scale=1.0,
                                )
                            nc.sync.dma_start(
                                ptb[:, il, :, :],
                                p_sb[:],
                                transpose=True,
                            )
                        # PV with [V_h | 1] stationary -> [O^T ; denom-row]
                        ot_ps = psC.tile([65, QBW], f32, tag="ot")
                        for kc in range(NKC):
                            nc.tensor.matmul(
                                ot_ps[:],
                                v_sb[:, kc, hh, 0:65],
                                ptb[:, :, kc, :],
                                start=(kc == 0),
                                stop=(kc == NKC - 1),
                            )
                        # recip of denom row, broadcast to 64 partitions
                        rrow = st.tile([1, QBW], f32, tag="rrow")
                        nc.vector.reciprocal(rrow[:], ot_ps[64:65, :])
                        rb = st.tile([64, QBW], f32, tag="rb")
                        nc.gpsimd.partition_broadcast(rb[:], rrow[:])
                        nc.vector.tensor_mul(
                            oT[r0:r0 + 64, p, :], ot_ps[0:64, :], rb[:]
                        )
                    for il in range(QBW // 128):
                        i = qb * 4 + il
                        y_sb = yp.tile([128, D], f32, tag="y")
                        for half in range(2):
                            yq = psA.tile([128, 512], f32, tag="mm")
                            for pp_ in range(4):
                                nc.tensor.matmul(
                                    yq[:],
                                    oT[:, pp_, il * 128:(il + 1) * 128],
                                    wor[:, pp_, half * 512:(half + 1) * 512],
                                    start=(pp_ == 0),
                                    stop=(pp_ == 3),
                                )
                            nc.scalar.copy(
                                y_sb[:, half * 512:(half + 1) * 512], yq[:])
                        nc.sync.dma_start(y_d[i * 128:(i + 1) * 128, :], y_sb[:])

    nc.compile()
    return nc


def _prep_inputs(x, mask, WQ, WK, WV, WO):
    xm = (x.astype(np.float32) * mask.astype(np.float32)[:, :, None])
    in_maps = []
    for c in range(8):
        b, g = c // 2, c % 2
        idx = np.array(
            [dk * H + (g * HL + hh) for hh in range(HL) for dk in range(DK)]
        )
        in_maps.append({
            "xT": np.ascontiguousarray(xm[b].T),
            "wq": np.ascontiguousarray(WQ[:, idx] / np.sqrt(DK)).astype(np.float32),
            "wk": np.ascontiguousarray(WK[:, idx]).astype(np.float32),
            "wv": np.ascontiguousarray(WV[:, idx]).astype(np.float32),
            "wo": np.ascontiguousarray(WO[g * DH:(g + 1) * DH, :]).astype(np.float32),
        })
    return in_maps


def kernel(x, mask, WQ, WK, WV, WO, _want_results=False, _trace=False):
    from concourse.bass_utils import run_bass_kernel_spmd

    if "nc" not in _cache:
        _cache["nc"] = _build()
    nc = _cache["nc"]
    in_maps = _prep_inputs(np.asarray(x), np.asarray(mask), np.asarray(WQ),
                           np.asarray(WK), np.asarray(WV), np.asarray(WO))
    res = run_bass_kernel_spmd(nc, in_maps, list(range(8)), trace=_trace)
    ys = [res.results[c]["y"] for c in range(8)]
    mk = np.asarray(mask).astype(np.float32)
    out = np.empty((B, S, D), np.float32)
    for b in range(B):
        out[b] = np.abs((ys[2 * b] + ys[2 * b + 1]) * mk[b][:, None])
    if _want_results:
        return out, res
    return out



# revision 2
# speedup vs baseline: 2.2878x; 2.2878x over previous
"""TRN2 Bass kernel: MultiHeadSelfAttention (B=4, S=2048, D=1024, H=16, DK=64).

Sharding: 8 cores = 4 batches x 2 head-groups (8 heads each).

Key optimization vs the dense version: the padding mask kills ~half the keys
(exp(-1e6) == 0 exactly in f32) and ~half the queries (output is multiplied
by the query mask), so the host compacts each batch to its valid rows
(max 1044 for this distribution) padded to SV=1152. All attention work
(QK, softmax, PV) shrinks ~3.2x and the projections ~1.8x, exactly.

Per core: QK in f32r (TF32), softmax via one wide reduce_max (negated) +
one wide exp(bias=-max) -> bf16 P, P^T via DMA-transpose, PV with [V|1]
stationary -> [O^T; denom], 1/denom broadcast, normalization fused into
the O^T write, output projection from O^T, partial Y out. Host sums the
two head-group partials, applies abs, and scatters to valid positions.
"""

import os
import numpy as np

B, S, D, H, DK = 4, 2048, 1024, 16, 64
HG = 2            # head groups (tensor-parallel)
HL = H // HG      # heads per core = 8
DH = HL * DK      # 512 per-core head width
KT = D // 128     # 8 contraction tiles
SV = 1152         # padded valid-row count (max valid ~1044 for this dist)
NT = SV // 128    # 9 tiles of 128 (q tiles == k chunks)
QBS = (512, 512, 128)   # q block widths (sum == SV)

_cache = {}


def _build():
    from concourse import bacc
    import concourse.mybir as mybir
    import concourse.tile as tile

    f32 = mybir.dt.float32
    f32r = mybir.dt.float32r
    bf16 = mybir.dt.bfloat16
    Exp = mybir.ActivationFunctionType.Exp
    AX = mybir.AxisListType.X

    nc = bacc.Bacc("TRN2", target_bir_lowering=False, debug=False, num_devices=8)

    xT_d = nc.dram_tensor("xT", [D, SV], f32, kind="ExternalInput")
    wq_d = nc.dram_tensor("wq", [D, DH], f32, kind="ExternalInput")
    wk_d = nc.dram_tensor("wk", [D, DH], f32, kind="ExternalInput")
    wv_d = nc.dram_tensor("wv", [D, DH], f32, kind="ExternalInput")
    wo_d = nc.dram_tensor("wo", [DH, D], f32, kind="ExternalInput")
    y_d = nc.dram_tensor("y", [SV, D], f32, kind="ExternalOutput")

    with tile.TileContext(nc) as tc:
        with (
            tc.tile_pool(name="persist", bufs=1) as pp,
            tc.tile_pool(name="psS", bufs=2, space="PSUM") as psS,
            tc.tile_pool(name="psSm", bufs=2, space="PSUM") as psSm,
        ):
            qT = pp.tile([128, 4, SV], f32r, tag="qT")
            kT = pp.tile([128, 4, SV], f32r, tag="kT")
            # V with a ones column per head: blocks of 66 = [V_h(64) | 1 | pad]
            v_sb = pp.tile([128, NT, HL, 66], bf16, tag="v")
            nc.gpsimd.memset(v_sb[:, :, :, 64:65], 1.0)
            wor = pp.tile([128, 4, D], f32r, tag="wor")
            nc.gpsimd.dma_start(wor[:], wo_d.rearrange("(t p) n -> p t n", p=128))

            # ---- phase 1: projections ----
            with (
                tc.tile_pool(name="ph1x", bufs=1) as px,
                tc.tile_pool(name="ph1w", bufs=10) as pw,
                tc.tile_pool(name="ph1wv", bufs=1) as pwv,
            ):
                xr = px.tile([128, KT, SV], f32r, tag="xr")
                nc.gpsimd.dma_start(
                    xr[:], xT_d.rearrange("(t p) s -> p t s", p=128)
                )
                wvr = pwv.tile([128, KT, DH], f32r, tag="wvr")
                nc.gpsimd.dma_start(
                    wvr[:], wv_d.rearrange("(t p) n -> p t n", p=128)
                )
                # Q^T / K^T: [dh-tile 128, SV] accumulated over 8 d-chunks
                for w_d, dst in ((wq_d, qT), (wk_d, kT)):
                    for p in range(4):
                        wchs = []
                        for k in range(KT):
                            wch = pw.tile([128, 128], f32r, tag="wch")
                            nc.gpsimd.dma_start(
                                wch[:],
                                w_d[k * 128:(k + 1) * 128, p * 128:(p + 1) * 128],
                            )
                            wchs.append(wch)
                        ps = psS.tile([128, SV], f32, tag="S")
                        n0 = 0
                        for nw in QBS:
                            for k in range(KT):
                                nc.tensor.matmul(
                                    ps[:, n0:n0 + nw],
                                    wchs[k][:],
                                    xr[:, k, n0:n0 + nw],
                                    start=(k == 0),
                                    stop=(k == KT - 1),
                                )
                            n0 += nw
                        nc.vector.tensor_copy(dst[:, p, :], ps[:])
                # V: [s-tile 128, 512] accumulated over 8 d-chunks
                for sc in range(NT):
                    psv = psSm.tile([128, 512], f32, tag="mm")
                    for k in range(KT):
                        nc.tensor.matmul(
                            psv[:],
                            xr[:, k, sc * 128:(sc + 1) * 128],
                            wvr[:, k, :],
                            start=(k == 0),
                            stop=(k == KT - 1),
                        )
                    nc.scalar.copy(
                        v_sb[:, sc, :, 0:64],
                        psv[:].rearrange("p (h w) -> p h w", w=64),
                    )

            # ---- phase 2: attention + output projection ----
            with (
                tc.tile_pool(name="ptb", bufs=2, space="SBUF") as ptbp,
                tc.tile_pool(name="pexp", bufs=3) as pexp,
                tc.tile_pool(name="stats", bufs=4) as st,
                tc.tile_pool(name="oTp", bufs=2) as oTp,
                tc.tile_pool(name="yp", bufs=3) as yp,
            ):
                i0 = 0
                for qb, qw in enumerate(QBS):
                    nil = qw // 128
                    oT = oTp.tile([128, 4, 512], f32r, tag="oT")
                    ptbs = []
                    # QK + softmax + transpose for all heads of this q block
                    for hh in range(HL):
                        p, r0 = hh // 2, (hh % 2) * 64
                        ptb = ptbp.tile([128, NT, 512], bf16, tag="ptb")
                        ptbs.append(ptb)
                        for il in range(nil):
                            i = i0 + il
                            sq = psS.tile([128, SV], f32, tag="S")
                            n0 = 0
                            for nw in QBS:
                                nc.tensor.matmul(
                                    sq[:, n0:n0 + nw],
                                    qT[r0:r0 + DK, p, i * 128:(i + 1) * 128],
                                    kT[r0:r0 + DK, p, n0:n0 + nw],
                                    start=True,
                                    stop=True,
                                )
                                n0 += nw
                            nm = st.tile([128, 1], f32, tag="nm")
                            nc.vector.tensor_reduce(
                                nm[:], sq[:], axis=AX,
                                op=mybir.AluOpType.max, negate=True,
                            )
                            p_sb = pexp.tile([128, SV], bf16, tag="p")
                            nc.scalar.activation(
                                p_sb[:], sq[:], Exp, bias=nm[:], scale=1.0,
                            )
                            nc.sync.dma_start(
                                ptb[:, :, il * 128:(il + 1) * 128],
                                p_sb[:],
                                transpose=True,
                            )
                    # PV with [V_h | 1] stationary -> [O^T ; denom-row]
                    for hh in range(HL):
                        p, r0 = hh // 2, (hh % 2) * 64
                        ot_ps = psSm.tile([65, 512], f32, tag="mm")
                        for kc in range(NT):
                            nc.tensor.matmul(
                                ot_ps[:, 0:qw],
                                v_sb[:, kc, hh, 0:65],
                                ptbs[hh][:, kc, 0:qw],
                                start=(kc == 0),
                                stop=(kc == NT - 1),
                            )
                        rrow = st.tile([1, 512], f32, tag="rrow")
                        nc.vector.reciprocal(rrow[:, 0:qw], ot_ps[64:65, 0:qw])
                        rb = st.tile([64, 512], f32, tag="rb")
                        nc.gpsimd.partition_broadcast(rb[:, 0:qw], rrow[:, 0:qw])
                        nc.vector.tensor_mul(
                            oT[r0:r0 + 64, p, 0:qw], ot_ps[0:64, 0:qw], rb[:, 0:qw]
                        )
                    # output projection for this q block
                    for il in range(nil):
                        i = i0 + il
                        y_sb = yp.tile([128, D], f32, tag="y")
                        for half in range(2):
                            yq = psSm.tile([128, 512], f32, tag="mm")
                            for pp_ in range(4):
                                nc.tensor.matmul(
                                    yq[:],
                                    oT[:, pp_, il * 128:(il + 1) * 128],
                                    wor[:, pp_, half * 512:(half + 1) * 512],
                                    start=(pp_ == 0),
                                    stop=(pp_ == 3),
                                )
                            nc.scalar.copy(
                                y_sb[:, half * 512:(half + 1) * 512], yq[:])
                        nc.sync.dma_start(y_d[i * 128:(i + 1) * 128, :], y_sb[:])
                    i0 += nil

    nc.compile()
    return nc


def _prep_inputs(x, mask, WQ, WK, WV, WO):
    idx_list = [np.nonzero(mask[b])[0] for b in range(B)]
    in_maps = []
    for c in range(8):
        b, g = c // 2, c % 2
        idx = idx_list[b]
        xc = np.zeros((SV, D), np.float32)
        xc[:len(idx)] = x[b][idx]
        perm = np.array(
            [dk * H + (g * HL + hh) for hh in range(HL) for dk in range(DK)]
        )
        in_maps.append({
            "xT": np.ascontiguousarray(xc.T),
            "wq": np.ascontiguousarray(WQ[:, perm] / np.sqrt(DK)).astype(np.float32),
            "wk": np.ascontiguousarray(WK[:, perm]).astype(np.float32),
            "wv": np.ascontiguousarray(WV[:, perm]).astype(np.float32),
            "wo": np.ascontiguousarray(WO[g * DH:(g + 1) * DH, :]).astype(np.float32),
        })
    return in_maps, idx_list


def _ref_fallback(x, mask, WQ, WK, WV, WO):
    # numpy fallback for masks with > SV valid rows in a batch (never the
    # case for the target distribution); keeps kernel() correct for any mask.
    out = np.empty((B, S, D), np.float32)
    for b in range(B):
        q = (x[b] @ WQ).reshape(S, DK, H).transpose(2, 0, 1)
        k = (x[b] @ WK).reshape(S, DK, H).transpose(2, 1, 0)
        s = (q @ k) / np.sqrt(DK) - (~mask[b]).astype(np.float32)[None, None, :] * 1e6
        s = s - s.max(axis=-1, keepdims=True)
        e = np.exp(s)
        p = e / e.sum(axis=-1, keepdims=True)
        v = (x[b] @ WV).reshape(S, DK, H).transpose(2, 0, 1)
        o = (p @ v).transpose(1, 0, 2).reshape(S, D)
        out[b] = np.abs((o @ WO) * mask[b].astype(np.float32)[:, None])
    return out


def kernel(x, mask, WQ, WK, WV, WO, _want_results=False, _trace=False):
    from concourse.bass_utils import run_bass_kernel_spmd

    x = np.asarray(x, dtype=np.float32)
    mask = np.asarray(mask).astype(bool)
    WQ, WK = np.asarray(WQ, np.float32), np.asarray(WK, np.float32)
    WV, WO = np.asarray(WV, np.float32), np.asarray(WO, np.float32)

    if max(int(mask[b].sum()) for b in range(B)) > SV:
        return _ref_fallback(x, mask, WQ, WK, WV, WO)

    if "nc" not in _cache:
        _cache["nc"] = _build()
    nc = _cache["nc"]
    in_maps, idx_list = _prep_inputs(x, mask, WQ, WK, WV, WO)
    res = run_bass_kernel_spmd(nc, in_maps, list(range(8)), trace=_trace)
    out = np.zeros((B, S, D), np.float32)
    for b in range(B):
        idx = idx_list[b]
        yb = res.results[2 * b]["y"][:len(idx)] + res.results[2 * b + 1]["y"][:len(idx)]
        out[b][idx] = np.abs(yb)
    if _want_results:
        return out, res
    return out


# revision 12
# speedup vs baseline: 2.4433x; 1.0679x over previous
"""TRN2 Bass kernel: MultiHeadSelfAttention (B=4, S=2048, D=1024, H=16, DK=64).

Sharding: 8 cores = 4 batches x 2 head-groups (8 heads each).

Key optimization vs the dense version: the padding mask kills ~half the keys
(exp(-1e6) == 0 exactly in f32) and ~half the queries (output is multiplied
by the query mask), so the host compacts each batch to its valid rows
(max 1044 for this distribution) padded to SV=1152. All attention work
(QK, softmax, PV) shrinks ~3.2x and the projections ~1.8x, exactly.

Per core: QK in f32r (TF32), softmax via one wide reduce_max (negated) +
one wide exp(bias=-max) -> bf16 P, P^T via DMA-transpose, PV with [V|1]
stationary -> [O^T; denom], 1/denom broadcast, normalization on gpsimd,
output projection from O^T, partial Y out. Host sums the two head-group
partials, applies abs, and scatters to valid positions.

The (qb, head) stream is software-pipelined: PV/output-projection for
head j runs while QK/softmax for head j+LAG streams, so PE never waits
on the DVE->Act->DMA softmax chain.
"""

import os
import numpy as np

B, S, D, H, DK = 4, 2048, 1024, 16, 64
HG = 2            # head groups (tensor-parallel)
HL = H // HG      # heads per core = 8
DH = HL * DK      # 512 per-core head width
KT = D // 128     # 8 contraction tiles
SV = 1152         # padded valid-row count (max valid ~1044 for this dist)
SVC = 1056        # trimmed compute width (>= max valid count, mult of 32)
NT = SV // 128    # 9 tiles of 128 (q tiles == k chunks)
QBS = (512, 512, 128)   # q block widths (sum == SV)
QBC = (512, 512, 32)    # q block valid widths (sum == SVC)
LAG = 3           # software pipeline depth in heads

_cache = {}


def _build():
    from concourse import bacc
    import concourse.mybir as mybir
    import concourse.tile as tile

    f32 = mybir.dt.float32
    f32r = mybir.dt.float32r
    bf16 = mybir.dt.bfloat16
    Exp = mybir.ActivationFunctionType.Exp
    AX = mybir.AxisListType.X

    nc = bacc.Bacc("TRN2", target_bir_lowering=False, debug=False, num_devices=8)

    xT_d = nc.dram_tensor("xT", [D, SV], f32, kind="ExternalInput")
    wq_d = nc.dram_tensor("wq", [D, DH], f32, kind="ExternalInput")
    wk_d = nc.dram_tensor("wk", [D, DH], f32, kind="ExternalInput")
    wv_d = nc.dram_tensor("wv", [D, DH], f32, kind="ExternalInput")
    wo_d = nc.dram_tensor("wo", [DH, D], f32, kind="ExternalInput")
    y_d = nc.dram_tensor("y", [SV, D], f32, kind="ExternalOutput")

    with tile.TileContext(nc) as tc:
        with (
            tc.tile_pool(name="persist", bufs=1) as pp,
            tc.tile_pool(name="psS", bufs=2, space="PSUM") as psS,
            tc.tile_pool(name="psSm", bufs=2, space="PSUM") as psSm,
            tc.tile_pool(name="ptbp", bufs=LAG + 1) as ptbp,
            tc.tile_pool(name="pexp", bufs=2) as pexp,
            tc.tile_pool(name="stats", bufs=3) as st,
            tc.tile_pool(name="oTp", bufs=2) as oTp,
            tc.tile_pool(name="yp", bufs=2) as yp,
            tc.tile_pool(name="ph1x", bufs=1) as px,
            tc.tile_pool(name="ph1w", bufs=2) as pw,
            tc.tile_pool(name="ph1wv", bufs=1) as pwv,
        ):
            qT = pp.tile([128, 4, SV], f32r, tag="qT")
            kT = pp.tile([128, 4, SV], f32r, tag="kT")
            # V with a ones column per head: blocks of 66 = [V_h(64) | 1 | pad]
            v_sb = pp.tile([128, NT, HL, 66], bf16, tag="v")
            nc.gpsimd.memset(v_sb[:, :, :, 64:65], 1.0)
            wor = pp.tile([128, 4, D], f32r, tag="wor")

            # ---- input loads (HWDGE queues) ----
            xr = px.tile([128, KT, SV], f32r, tag="xr")
            nc.gpsimd.dma_start(xr[:], xT_d.rearrange("(t p) s -> p t s", p=128))
            wvr = pwv.tile([128, KT, DH], f32r, tag="wvr")
            nc.gpsimd.dma_start(wvr[:], wv_d.rearrange("(t p) n -> p t n", p=128))
            nc.gpsimd.dma_start(wor[:], wo_d.rearrange("(t p) n -> p t n", p=128))

            def proj_qk(w_d, dst, p):
                wchs = pw.tile([128, KT, 128], f32r, tag="wch")
                nc.gpsimd.dma_start(
                    wchs[:],
                    w_d[:, p * 128:(p + 1) * 128].rearrange(
                        "(t p) n -> p t n", p=128),
                )
                ps = psS.tile([128, SV], f32, tag="S")
                n0 = 0
                for nw in QBS:
                    for k in range(KT):
                        nc.tensor.matmul(
                            ps[:, n0:n0 + nw],
                            wchs[:, k, :],
                            xr[:, k, n0:n0 + nw],
                            start=(k == 0),
                            stop=(k == KT - 1),
                        )
                    n0 += nw
                nc.vector.tensor_copy(dst[:, p, :], ps[:])

            def proj_v(sc):
                psv = psSm.tile([128, 512], f32, tag="mm")
                for k in range(KT):
                    nc.tensor.matmul(
                        psv[:],
                        xr[:, k, sc * 128:(sc + 1) * 128],
                        wvr[:, k, :],
                        start=(k == 0),
                        stop=(k == KT - 1),
                    )
                nc.scalar.copy(
                    v_sb[:, sc, :, 0:64],
                    psv[:].rearrange("p (h w) -> p h w", w=64),
                )

            # Q/K projections for all 4 partition tiles (dh tiles)
            for p in range(4):
                proj_qk(wq_d, qT, p)
                proj_qk(wk_d, kT, p)

            # ---- attention stream, software-pipelined over (qb, head) ----
            qb_off = [0, 512, 1024]
            oTs = {}

            def emit_qk(qb, hh):
                """QK + softmax + P^T for all q tiles of block qb, head hh."""
                p, r0 = hh // 2, (hh % 2) * 64
                qw = QBS[qb]
                ptb = ptbp.tile([128, NT, 512], bf16, tag="ptb")
                for il in range(qw // 128):
                    i = qb_off[qb] // 128 + il
                    sq = psS.tile([128, SV], f32, tag="S")
                    n0 = 0
                    for nw in QBS:
                        nc.tensor.matmul(
                            sq[:, n0:n0 + nw],
                            qT[r0:r0 + DK, p, i * 128:(i + 1) * 128],
                            kT[r0:r0 + DK, p, n0:n0 + nw],
                            start=True,
                            stop=True,
                        )
                        n0 += nw
                    nm = st.tile([128, 1], f32, tag="nm")
                    nc.vector.tensor_reduce(
                        nm[:], sq[:, 0:SVC], axis=AX,
                        op=mybir.AluOpType.max, negate=True,
                    )
                    p_sb = pexp.tile([128, SV], bf16, tag="p")
                    nc.scalar.activation(
                        p_sb[:], sq[:], Exp, bias=nm[:], scale=1.0,
                    )
                    nc.sync.dma_start(
                        ptb[:, :, il * 128:(il + 1) * 128],
                        p_sb[:],
                        transpose=True,
                    )
                return ptb

            def emit_pv(qb, hh, ptb):
                p, r0 = hh // 2, (hh % 2) * 64
                qw = QBC[qb]
                if hh == 0:
                    oTs[qb] = oTp.tile([128, 4, 512], f32r, tag="oT",
                                       name=f"oT{qb}")
                ot_ps = psSm.tile([65, 512], f32, tag="mm")
                for kc in range(NT):
                    nc.tensor.matmul(
                        ot_ps[:, 0:qw],
                        v_sb[:, kc, hh, 0:65],
                        ptb[:, kc, 0:qw],
                        start=(kc == 0),
                        stop=(kc == NT - 1),
                    )
                rrow = st.tile([1, 512], f32, tag="rrow")
                nc.vector.reciprocal(rrow[:, 0:qw], ot_ps[64:65, 0:qw])
                rb = st.tile([64, 512], f32, tag="rb")
                nc.gpsimd.partition_broadcast(rb[:, 0:qw], rrow[:, 0:qw])
                nc.vector.tensor_mul(
                    oTs[qb][r0:r0 + 64, p, 0:qw], ot_ps[0:64, 0:qw],
                    rb[:, 0:qw],
                )

            def emit_op(qb):
                """Output projection for q block qb."""
                qw = QBS[qb]
                oT = oTs[qb]
                for il in range(qw // 128):
                    i = qb_off[qb] // 128 + il
                    y_sb = yp.tile([128, D], f32, tag="y")
                    for half in range(2):
                        yq = psSm.tile([128, 512], f32, tag="mm")
                        for pp_ in range(4):
                            nc.tensor.matmul(
                                yq[:],
                                oT[:, pp_, il * 128:(il + 1) * 128],
                                wor[:, pp_, half * 512:(half + 1) * 512],
                                start=(pp_ == 0),
                                stop=(pp_ == 3),
                            )
                        nc.scalar.copy(
                            y_sb[:, half * 512:(half + 1) * 512], yq[:])
                    nc.sync.dma_start(y_d[i * 128:(i + 1) * 128, :], y_sb[:])

            stream = [(qb, hh) for qb in range(3) for hh in range(HL)]
            ptbs = {}
            op_pending = []  # (qb, countdown)
            for idx, (qb, hh) in enumerate(stream):
                ptbs[(qb, hh)] = emit_qk(qb, hh)
                # V projection interleaved early in the stream (needs to be
                # done before the first PV, i.e. before idx reaches LAG)
                if idx == 0:
                    for sc in range(NT):
                        proj_v(sc)
                k = idx - LAG
                if k >= 0:
                    qbk, hhk = stream[k]
                    emit_pv(qbk, hhk, ptbs.pop((qbk, hhk)))
                    if hhk == HL - 1:
                        op_pending.append([qbk, 2])
                for ent in list(op_pending):
                    ent[1] -= 1
                    if ent[1] <= 0:
                        emit_op(ent[0])
                        op_pending.remove(ent)
            for k in range(len(stream) - LAG, len(stream)):
                qbk, hhk = stream[k]
                emit_pv(qbk, hhk, ptbs.pop((qbk, hhk)))
            emit_op(2)

    nc.compile()
    return nc


def _prep_inputs(x, mask, WQ, WK, WV, WO):
    idx_list = [np.nonzero(mask[b])[0] for b in range(B)]
    in_maps = []
    for c in range(8):
        b, g = c // 2, c % 2
        idx = idx_list[b]
        xc = np.zeros((SV, D), np.float32)
        xc[:len(idx)] = x[b][idx]
        perm = np.array(
            [dk * H + (g * HL + hh) for hh in range(HL) for dk in range(DK)]
        )
        in_maps.append({
            "xT": np.ascontiguousarray(xc.T),
            "wq": np.ascontiguousarray(WQ[:, perm] / np.sqrt(DK)).astype(np.float32),
            "wk": np.ascontiguousarray(WK[:, perm]).astype(np.float32),
            "wv": np.ascontiguousarray(WV[:, perm]).astype(np.float32),
            "wo": np.ascontiguousarray(WO[g * DH:(g + 1) * DH, :]).astype(np.float32),
        })
    return in_maps, idx_list


def _ref_fallback(x, mask, WQ, WK, WV, WO):
    # numpy fallback for masks with > SVC valid rows in a batch (never the
    # case for the target distribution); keeps kernel() correct for any mask.
    out = np.empty((B, S, D), np.float32)
    for b in range(B):
        q = (x[b] @ WQ).reshape(S, DK, H).transpose(2, 0, 1)
        k = (x[b] @ WK).reshape(S, DK, H).transpose(2, 1, 0)
        s = (q @ k) / np.sqrt(DK) - (~mask[b]).astype(np.float32)[None, None, :] * 1e6
        s = s - s.max(axis=-1, keepdims=True)
        e = np.exp(s)
        p = e / e.sum(axis=-1, keepdims=True)
        v = (x[b] @ WV).reshape(S, DK, H).transpose(2, 0, 1)
        o = (p @ v).transpose(1, 0, 2).reshape(S, D)
        out[b] = np.abs((o @ WO) * mask[b].astype(np.float32)[:, None])
    return out


def kernel(x, mask, WQ, WK, WV, WO, _want_results=False, _trace=False):
    from concourse.bass_utils import run_bass_kernel_spmd

    x = np.asarray(x, dtype=np.float32)
    mask = np.asarray(mask).astype(bool)
    WQ, WK = np.asarray(WQ, np.float32), np.asarray(WK, np.float32)
    WV, WO = np.asarray(WV, np.float32), np.asarray(WO, np.float32)

    if max(int(mask[b].sum()) for b in range(B)) > SVC:
        return _ref_fallback(x, mask, WQ, WK, WV, WO)

    if "nc" not in _cache:
        _cache["nc"] = _build()
    nc = _cache["nc"]
    in_maps, idx_list = _prep_inputs(x, mask, WQ, WK, WV, WO)
    res = run_bass_kernel_spmd(nc, in_maps, list(range(8)), trace=_trace)
    out = np.zeros((B, S, D), np.float32)
    for b in range(B):
        idx = idx_list[b]
        yb = res.results[2 * b]["y"][:len(idx)] + res.results[2 * b + 1]["y"][:len(idx)]
        out[b][idx] = np.abs(yb)
    if _want_results:
        return out, res
    return out
